# revision 14
# baseline (speedup 1.0000x reference)
"""GNN message passing (gather + scatter-add) on 8 trn2 NeuronCores.

Strategy: shard by destination node range (12500 nodes per core), then
process 512-dst PSUM windows (25 per core). Host sorts each core's edges
into (window, src-chunk) regions — src indices are chunk-relative so they
fit dma_gather's int16 index planes (4 chunks of 25000 x-rows) and are
sorted by src within a region for HBM row locality. Per region:
  1. ONE dma_gather fetches all its edges' 256B bf16 x-rows in one Q7
     dispatch — both hardware walls (Q7 descriptor generation and the
     random-256B HBM drain) cost ~8ns per descriptor, so region capacity
     (a uniform SPMD constant, ~15% padding) sets the floor,
  2. one DVE compare builds the [slot, 512] one-hot dst-selection,
  3. per 128-slot block, matmul psum[feat, dst512] += msg^T @ sel
     (dst on the free dim so a window needs one PSUM bank; output is
     feature-major and the host transposes at the end),
  4. psum -> SBUF copy, streamed to DRAM.
Pad slots keep dst -1 (sel column zero); msg buffers are memset once so
stale pad rows stay finite. No collective; each core owns its dst slice.
"""

import os
import sys

import numpy as np
import ml_dtypes

for _p in ("/opt/trn_rl_repo",):
    if _p not in sys.path:
        sys.path.insert(0, _p)

from concourse import bass, mybir, tile, bacc  # noqa: E402
from concourse.bass_utils import run_bass_kernel_spmd  # noqa: E402

P = 128
D = 128
W = 512  # dsts per PSUM window
N_NODES = 100000
N_CORES = 8
NODES_PER_CORE = N_NODES // N_CORES  # 12500
N_CHUNKS = 4
CHUNK = 25000  # int16 gather indices must stay < 32768


def build_program(n_nodes, chunk, n_chunks, n_win, cap, num_devices):
    """One SPMD program; per-core inputs idxT/dstT select the edges.

    cap = 128-slot blocks per (window, chunk) region, uniform across
    windows and cores so all 8 cores share the program.
    """
    kstat = cap * P  # static idx capacity per region
    n_reg = n_win * n_chunks
    tot = n_reg * kstat  # total slots per core
    nmm = tot // P  # total 128-edge blocks

    nc = bacc.Bacc(
        "TRN2", target_bir_lowering=False, debug=False, num_devices=num_devices
    )
    xp = nc.dram_tensor(
        "xp", [n_nodes, D], mybir.dt.bfloat16, kind="ExternalInput"
    ).ap()
    idxT = nc.dram_tensor(
        "idxT", [P, tot // 16], mybir.dt.int16, kind="ExternalInput"
    ).ap()
    dstT = nc.dram_tensor("dstT", [P, nmm], mybir.dt.int16, kind="ExternalInput").ap()
    iota = nc.dram_tensor("iota", [P, W], mybir.dt.int16, kind="ExternalInput").ap()
    out = nc.dram_tensor(
        "out", [P, n_win * W], mybir.dt.float32, kind="ExternalOutput"
    ).ap()

    with tile.TileContext(nc) as tc:
        with tc.tile_pool(name="sb", bufs=1) as pool, tc.tile_pool(
            name="ps", bufs=1, space="PSUM"
        ) as psp:
            ix = pool.tile([P, tot // 16], mybir.dt.int16)
            dst_sb = pool.tile([P, nmm], mybir.dt.int16)
            iot = pool.tile([P, W], mybir.dt.int16)
            nc.sync.dma_start(out=ix[:], in_=idxT[:])
            nc.sync.dma_start(out=dst_sb[:], in_=dstT[:])
            nc.sync.dma_start(out=iot[:], in_=iota[:])

            nbuf = 2
            msg = [
                pool.tile([P, n_chunks * cap, D], mybir.dt.bfloat16, name=f"msg{i}")
                for i in range(nbuf)
            ]
            slb = [
                pool.tile([P, cap, W], mybir.dt.bfloat16, name=f"slb{i}")
                for i in range(nbuf)
            ]
            stg = [
                pool.tile([P, W], mybir.dt.float32, name=f"stg{i}") for i in range(nbuf)
            ]
            pst = [
                psp.tile([P, W], dtype=mybir.dt.float32, space="PSUM", name=f"pst{i}")
                for i in range(nbuf)
            ]
            for w in range(n_win):
                i = w % nbuf
                mg, ps = msg[i], pst[i]
                for k in range(n_chunks):
                    r = w * n_chunks + k
                    # pad slots carry idx 0 (harmless row; sel column is 0):
                    # a runtime num_idxs_reg register would skip them, but
                    # that path wedges the exec unit on HW.
                    nc.gpsimd.dma_gather(
                        mg[:, k * cap : (k + 1) * cap, :],
                        xp[k * chunk : (k + 1) * chunk, :],
                        ix[:, r * kstat // 16 : (r + 1) * kstat // 16],
                        kstat,
                        kstat,
                        D,
                        single_packet=False,
                    )
                for k in range(n_chunks):
                    r = w * n_chunks + k
                    sl = slb[r % nbuf]
                    nc.vector.tensor_tensor(
                        out=sl[:],
                        in0=dst_sb[:, r * cap : (r + 1) * cap][
                            :, :, None
                        ].to_broadcast([P, cap, W]),
                        in1=iot[:, None, :].to_broadcast([P, cap, W]),
                        op=mybir.AluOpType.is_equal,
                    )
                    for b in range(cap):
                        nc.tensor.matmul(
                            out=ps[:],
                            lhsT=mg[:, k * cap + b, :],
                            rhs=sl[:, b, :],
                            start=(k == 0 and b == 0),
                            stop=(k == n_chunks - 1 and b == cap - 1),
                        )
                sg = stg[i]
                nc.scalar.copy(sg[:], ps[:])
                nc.sync.dma_start(out=out[:, w * W : (w + 1) * W], in_=sg[:])
    nc.compile()
    return nc


def compute_cap(src, dst, n_cores, nodes_per_core, chunk, n_chunks, n_win):
    """Blocks needed for the fullest (core, window, chunk) region."""
    src = np.asarray(src, dtype=np.int64)
    dst = np.asarray(dst, dtype=np.int64)
    core = dst // nodes_per_core
    w = (dst - core * nodes_per_core) // W
    k = src // chunk
    rid = (core * n_win + w) * n_chunks + k
    counts = np.bincount(rid, minlength=n_cores * n_win * n_chunks)
    return int(-(-counts.max() // P))


def prep_core(src, dst, core, cap, n_win, nodes_per_core, chunk, n_chunks):
    """Bin one core's edges into (window, chunk) region planes.

    Slot order inside a region: ascending src (HBM row locality); real
    edges first, then idx-0 pads. Returns idxT [128, tot/16] int16
    (gather plane: slot s -> partition s%16 replicated over the 8 Q7
    groups, col s/16) and dstT [128, tot/128] int16 (slot s -> partition
    s%128, block s/128; window-relative dst, -1 on pads).
    """
    kstat = cap * P
    n_reg = n_win * n_chunks
    tot = n_reg * kstat

    lo = core * nodes_per_core
    m = (dst >= lo) & (dst < lo + nodes_per_core)
    es = src[m].astype(np.int64)
    ed = (dst[m] - lo).astype(np.int64)
    w = ed // W
    k = es // chunk
    srel = es - k * chunk
    drel = ed - w * W
    rid = (w * n_chunks + k).astype(np.int64)

    order = np.lexsort((srel, rid))
    rid, srel, drel = rid[order], srel[order], drel[order]
    counts = np.bincount(rid, minlength=n_reg)
    starts = np.zeros(n_reg, dtype=np.int64)
    starts[1:] = np.cumsum(counts)[:-1]
    pos = np.arange(len(rid)) - starts[rid]
    slot = rid * kstat + pos

    idxflat = np.zeros(tot, np.int16)  # pad slots gather row 0 of their chunk
    dstflat = np.full(tot, -1, np.int16)
    idxflat[slot] = srel
    dstflat[slot] = drel
    idx16 = idxflat.reshape(tot // 16, 16).T  # [16, tot/16]
    idxT = np.ascontiguousarray(np.tile(idx16, (8, 1)))
    dstT = np.ascontiguousarray(dstflat.reshape(tot // P, P).T)
    return idxT, dstT


_cache = {}


def kernel(x, edge_index):
    x = np.asarray(x, dtype=np.float32)
    edge_index = np.asarray(edge_index)
    src = edge_index[0].astype(np.int64)
    dst = edge_index[1].astype(np.int64)

    n_win = -(-NODES_PER_CORE // W)  # 25
    cap = compute_cap(src, dst, N_CORES, NODES_PER_CORE, CHUNK, N_CHUNKS, n_win)

    key = (N_NODES, n_win, cap)
    if key not in _cache:
        _cache[key] = build_program(N_NODES, CHUNK, N_CHUNKS, n_win, cap, N_CORES)
    nc = _cache[key]

    xp = np.ascontiguousarray(x.astype(ml_dtypes.bfloat16))
    iota = np.tile(np.arange(W, dtype=np.int16), (P, 1))
    in_maps = []
    for c in range(N_CORES):
        idxT, dstT = prep_core(
            src, dst, c, cap, n_win, NODES_PER_CORE, CHUNK, N_CHUNKS
        )
        in_maps.append({"xp": xp, "idxT": idxT, "dstT": dstT, "iota": iota})

    trace = bool(int(os.environ.get("KERNEL_TRACE", "0")))
    res = run_bass_kernel_spmd(
        nc, in_maps, core_ids=list(range(N_CORES)), trace=trace
    )
    if trace:
        kernel.last_results = res
    outs = [
        np.ascontiguousarray(res.results[c]["out"][:, :NODES_PER_CORE].T)
        for c in range(N_CORES)
    ]
    return np.ascontiguousarray(np.concatenate(outs, axis=0))


# revision 17
# speedup vs baseline: 1.1588x; 1.1588x over previous
"""GNN message passing (gather + scatter-add) on 8 trn2 NeuronCores.

Strategy: shard by destination node range (12500 nodes per core), then
process 512-dst PSUM windows (25 per core). Host sorts each core's edges
into (window, src-chunk) regions — src indices are chunk-relative so they
fit dma_gather's int16 index planes (4 chunks of 25000 x-rows) and are
sorted by src within a region for HBM row locality. Per region:
  1. ONE dma_gather fetches all its edges' 256B bf16 x-rows in one Q7
     dispatch — both hardware walls (Q7 descriptor generation and the
     random-256B HBM drain) cost ~8ns per descriptor, so region capacity
     (a uniform SPMD constant, ~15% padding) sets the floor,
  2. one DVE compare builds the [slot, 512] one-hot dst-selection,
  3. per 128-slot block, matmul psum[feat, dst512] += msg^T @ sel
     (dst on the free dim so a window needs one PSUM bank; output is
     feature-major and the host transposes at the end),
  4. psum -> SBUF copy, streamed to DRAM.
Pad slots keep dst -1 (sel column zero); msg buffers are memset once so
stale pad rows stay finite. No collective; each core owns its dst slice.
"""

import os
import sys

import numpy as np
import ml_dtypes

for _p in ("/opt/trn_rl_repo",):
    if _p not in sys.path:
        sys.path.insert(0, _p)

from concourse import bass, mybir, tile, bacc  # noqa: E402
from concourse.bass_utils import run_bass_kernel_spmd  # noqa: E402

P = 128
D = 128
W = 512  # dsts per PSUM window
N_NODES = 100000
N_CORES = 8
NODES_PER_CORE = N_NODES // N_CORES  # 12500
N_CHUNKS = 4
CHUNK = 25000  # int16 gather indices must stay < 32768


def build_program(n_nodes, chunk, n_chunks, n_win, cap, num_devices):
    """One SPMD program; per-core inputs idxT/dstT select the edges.

    cap = 128-slot blocks per (window, chunk) region, uniform across
    windows and cores so all 8 cores share the program.
    """
    kstat = cap * P  # static idx capacity per region
    n_reg = n_win * n_chunks
    tot = n_reg * kstat  # total slots per core
    nmm = tot // P  # total 128-edge blocks

    nc = bacc.Bacc(
        "TRN2", target_bir_lowering=False, debug=False, num_devices=num_devices
    )
    xp = nc.dram_tensor(
        "xp", [n_nodes, D], mybir.dt.bfloat16, kind="ExternalInput"
    ).ap()
    idxT = nc.dram_tensor(
        "idxT", [P, tot // 16], mybir.dt.int16, kind="ExternalInput"
    ).ap()
    dstT = nc.dram_tensor("dstT", [P, nmm], mybir.dt.int16, kind="ExternalInput").ap()
    iota = nc.dram_tensor("iota", [P, W], mybir.dt.int16, kind="ExternalInput").ap()
    out = nc.dram_tensor(
        "out", [P, n_win * W], mybir.dt.float32, kind="ExternalOutput"
    ).ap()

    with tile.TileContext(nc) as tc:
        with tc.tile_pool(name="sb", bufs=1) as pool, tc.tile_pool(
            name="ps", bufs=1, space="PSUM"
        ) as psp:
            ix = pool.tile([P, tot // 16], mybir.dt.int16)
            dst_sb = pool.tile([P, nmm], mybir.dt.int16)
            iot = pool.tile([P, W], mybir.dt.int16)
            nc.sync.dma_start(out=ix[:], in_=idxT[:])
            nc.sync.dma_start(out=dst_sb[:], in_=dstT[:])
            nc.sync.dma_start(out=iot[:], in_=iota[:])

            nbuf = 2
            msg = [
                pool.tile([P, n_chunks * cap, D], mybir.dt.bfloat16, name=f"msg{i}")
                for i in range(nbuf)
            ]
            slb = [
                pool.tile([P, cap, W], mybir.dt.bfloat16, name=f"slb{i}")
                for i in range(nbuf)
            ]
            stg = [
                pool.tile([P, W], mybir.dt.float32, name=f"stg{i}") for i in range(nbuf)
            ]
            pst = [
                psp.tile([P, W], dtype=mybir.dt.float32, space="PSUM", name=f"pst{i}")
                for i in range(nbuf)
            ]
            for w in range(n_win):
                i = w % nbuf
                mg, ps = msg[i], pst[i]
                for k in range(n_chunks):
                    r = w * n_chunks + k
                    # pad slots carry idx 0 (harmless row; sel column is 0):
                    # a runtime num_idxs_reg register would skip them, but
                    # that path wedges the exec unit on HW.
                    nc.gpsimd.dma_gather(
                        mg[:, k * cap : (k + 1) * cap, :],
                        xp[k * chunk : (k + 1) * chunk, :],
                        ix[:, r * kstat // 16 : (r + 1) * kstat // 16],
                        kstat,
                        kstat,
                        D,
                        single_packet=False,
                    )
                for k in range(n_chunks):
                    r = w * n_chunks + k
                    sl = slb[r % nbuf]
                    nc.vector.tensor_tensor(
                        out=sl[:],
                        in0=dst_sb[:, r * cap : (r + 1) * cap][
                            :, :, None
                        ].to_broadcast([P, cap, W]),
                        in1=iot[:, None, :].to_broadcast([P, cap, W]),
                        op=mybir.AluOpType.is_equal,
                    )
                    for b in range(cap):
                        nc.tensor.matmul(
                            out=ps[:],
                            lhsT=mg[:, k * cap + b, :],
                            rhs=sl[:, b, :],
                            start=(k == 0 and b == 0),
                            stop=(k == n_chunks - 1 and b == cap - 1),
                        )
                sg = stg[i]
                nc.scalar.copy(sg[:], ps[:])
                nc.sync.dma_start(out=out[:, w * W : (w + 1) * W], in_=sg[:])
    nc.compile()
    return nc


def compute_cap(src, dst, n_cores, nodes_per_core, chunk, n_chunks, n_win):
    """Blocks needed for the fullest (core, window, chunk) region."""
    src = np.asarray(src, dtype=np.int64)
    dst = np.asarray(dst, dtype=np.int64)
    core = dst // nodes_per_core
    w = (dst - core * nodes_per_core) // W
    k = src // chunk
    rid = (core * n_win + w) * n_chunks + k
    counts = np.bincount(rid, minlength=n_cores * n_win * n_chunks)
    return int(-(-counts.max() // P))


def prep_core(src, dst, core, cap, n_win, nodes_per_core, chunk, n_chunks):
    """Bin one core's edges into (window, chunk) region planes.

    Slot order inside a region: ascending src (HBM row locality); real
    edges first, then idx-0 pads. Returns idxT [128, tot/16] int16
    (gather plane: slot s -> partition s%16 replicated over the 8 Q7
    groups, col s/16) and dstT [128, tot/128] int16 (slot s -> partition
    s%128, block s/128; window-relative dst, -1 on pads).
    """
    kstat = cap * P
    n_reg = n_win * n_chunks
    tot = n_reg * kstat

    lo = core * nodes_per_core
    m = (dst >= lo) & (dst < lo + nodes_per_core)
    es = src[m].astype(np.int64)
    ed = (dst[m] - lo).astype(np.int64)
    w = ed // W
    k = es // chunk
    srel = es - k * chunk
    drel = ed - w * W
    rid = (w * n_chunks + k).astype(np.int64)

    order = np.argsort(rid, kind="stable")  # unsorted within region: sorted-by-src caused HBM bank conflicts across the 16 SDMA engines
    rid, srel, drel = rid[order], srel[order], drel[order]
    counts = np.bincount(rid, minlength=n_reg)
    starts = np.zeros(n_reg, dtype=np.int64)
    starts[1:] = np.cumsum(counts)[:-1]
    pos = np.arange(len(rid)) - starts[rid]
    slot = rid * kstat + pos

    idxflat = np.zeros(tot, np.int16)  # pad slots gather row 0 of their chunk
    dstflat = np.full(tot, -1, np.int16)
    idxflat[slot] = srel
    dstflat[slot] = drel
    idx16 = idxflat.reshape(tot // 16, 16).T  # [16, tot/16]
    idxT = np.ascontiguousarray(np.tile(idx16, (8, 1)))
    dstT = np.ascontiguousarray(dstflat.reshape(tot // P, P).T)
    return idxT, dstT


_cache = {}


def kernel(x, edge_index):
    x = np.asarray(x, dtype=np.float32)
    edge_index = np.asarray(edge_index)
    src = edge_index[0].astype(np.int64)
    dst = edge_index[1].astype(np.int64)

    n_win = -(-NODES_PER_CORE // W)  # 25
    cap = compute_cap(src, dst, N_CORES, NODES_PER_CORE, CHUNK, N_CHUNKS, n_win)

    key = (N_NODES, n_win, cap)
    if key not in _cache:
        _cache[key] = build_program(N_NODES, CHUNK, N_CHUNKS, n_win, cap, N_CORES)
    nc = _cache[key]

    xp = np.ascontiguousarray(x.astype(ml_dtypes.bfloat16))
    iota = np.tile(np.arange(W, dtype=np.int16), (P, 1))
    in_maps = []
    for c in range(N_CORES):
        idxT, dstT = prep_core(
            src, dst, c, cap, n_win, NODES_PER_CORE, CHUNK, N_CHUNKS
        )
        in_maps.append({"xp": xp, "idxT": idxT, "dstT": dstT, "iota": iota})

    trace = bool(int(os.environ.get("KERNEL_TRACE", "0")))
    res = run_bass_kernel_spmd(
        nc, in_maps, core_ids=list(range(N_CORES)), trace=trace
    )
    if trace:
        kernel.last_results = res
    outs = [
        np.ascontiguousarray(res.results[c]["out"][:, :NODES_PER_CORE].T)
        for c in range(N_CORES)
    ]
    return np.ascontiguousarray(np.concatenate(outs, axis=0))


# revision 18
# speedup vs baseline: 1.2759x; 1.1010x over previous
"""GNN message passing (gather + scatter-add) on 8 trn2 NeuronCores.

Strategy: shard by destination node range (12500 nodes per core), then
process 512-dst PSUM windows (25 per core). Host sorts each core's edges
into (window, src-chunk) regions — src indices are chunk-relative so they
fit dma_gather's int16 index planes (4 chunks of 25000 x-rows) and are
sorted by src within a region for HBM row locality. Per region:
  1. ONE dma_gather fetches all its edges' 256B bf16 x-rows in one Q7
     dispatch — both hardware walls (Q7 descriptor generation and the
     random-256B HBM drain) cost ~8ns per descriptor, so region capacity
     (a uniform SPMD constant, ~15% padding) sets the floor,
  2. one DVE compare builds the [slot, 512] one-hot dst-selection,
  3. per 128-slot block, matmul psum[feat, dst512] += msg^T @ sel
     (dst on the free dim so a window needs one PSUM bank; output is
     feature-major and the host transposes at the end),
  4. psum -> SBUF copy, streamed to DRAM.
Pad slots keep dst -1 (sel column zero); msg buffers are memset once so
stale pad rows stay finite. No collective; each core owns its dst slice.
"""

import os
import sys

import numpy as np
import ml_dtypes

for _p in ("/opt/trn_rl_repo",):
    if _p not in sys.path:
        sys.path.insert(0, _p)

from concourse import bass, mybir, tile, bacc  # noqa: E402
from concourse.bass_utils import run_bass_kernel_spmd  # noqa: E402

P = 128
D = 128
W = 512  # dsts per PSUM window
N_NODES = 100000
N_CORES = 8
NODES_PER_CORE = N_NODES // N_CORES  # 12500
N_CHUNKS = 4
CHUNK = 25000  # int16 gather indices must stay < 32768


def build_program(n_nodes, chunk, n_chunks, n_win, caps, num_devices):
    """One SPMD program; per-core inputs idxT/dstT select the edges.

    caps[w, k] = 128-slot blocks of (window w, chunk k)'s region — the
    fullest core's need, uniform across cores so all 8 share the program.
    """
    caps = np.asarray(caps)
    n_reg = n_win * n_chunks
    creg = caps.reshape(n_reg)
    roff = np.zeros(n_reg + 1, dtype=np.int64)  # region block offsets
    roff[1:] = np.cumsum(creg)
    nmm = int(roff[-1])  # total 128-edge blocks
    tot = nmm * P  # total slots per core
    capm = int(caps.sum(axis=1).max())  # msg blocks per window (max)
    cap1 = int(caps.max())  # sel blocks per region (max)

    nc = bacc.Bacc(
        "TRN2", target_bir_lowering=False, debug=False, num_devices=num_devices
    )
    xp = nc.dram_tensor(
        "xp", [n_nodes, D], mybir.dt.bfloat16, kind="ExternalInput"
    ).ap()
    idxT = nc.dram_tensor(
        "idxT", [P, tot // 16], mybir.dt.int16, kind="ExternalInput"
    ).ap()
    dstT = nc.dram_tensor("dstT", [P, nmm], mybir.dt.int16, kind="ExternalInput").ap()
    iota = nc.dram_tensor("iota", [P, W], mybir.dt.int16, kind="ExternalInput").ap()
    out = nc.dram_tensor(
        "out", [P, n_win * W], mybir.dt.float32, kind="ExternalOutput"
    ).ap()

    with tile.TileContext(nc) as tc:
        with tc.tile_pool(name="sb", bufs=1) as pool, tc.tile_pool(
            name="ps", bufs=1, space="PSUM"
        ) as psp:
            ix = pool.tile([P, tot // 16], mybir.dt.int16)
            dst_sb = pool.tile([P, nmm], mybir.dt.int16)
            iot = pool.tile([P, W], mybir.dt.int16)
            nc.sync.dma_start(out=ix[:], in_=idxT[:])
            nc.sync.dma_start(out=dst_sb[:], in_=dstT[:])
            nc.sync.dma_start(out=iot[:], in_=iota[:])

            nbuf = 2
            msg = [
                pool.tile([P, capm, D], mybir.dt.bfloat16, name=f"msg{i}")
                for i in range(nbuf)
            ]
            slb = [
                pool.tile([P, cap1, W], mybir.dt.bfloat16, name=f"slb{i}")
                for i in range(nbuf)
            ]
            stg = [
                pool.tile([P, W], mybir.dt.float32, name=f"stg{i}") for i in range(nbuf)
            ]
            pst = [
                psp.tile([P, W], dtype=mybir.dt.float32, space="PSUM", name=f"pst{i}")
                for i in range(nbuf)
            ]
            for w in range(n_win):
                i = w % nbuf
                mg, ps = msg[i], pst[i]
                wb0 = int(roff[w * n_chunks])  # window's first global block
                for k in range(n_chunks):
                    r = w * n_chunks + k
                    ck = int(creg[r])
                    b0 = int(roff[r]) - wb0
                    # pad slots carry idx 0 (harmless row; sel column is 0):
                    # a runtime num_idxs_reg register would skip them, but
                    # that path wedges the exec unit on HW.
                    nc.gpsimd.dma_gather(
                        mg[:, b0 : b0 + ck, :],
                        xp[k * chunk : (k + 1) * chunk, :],
                        ix[:, int(roff[r]) * P // 16 : int(roff[r + 1]) * P // 16],
                        ck * P,
                        ck * P,
                        D,
                        single_packet=False,
                    )
                for k in range(n_chunks):
                    r = w * n_chunks + k
                    ck = int(creg[r])
                    b0 = int(roff[r]) - wb0
                    sl = slb[r % nbuf]
                    nc.vector.tensor_tensor(
                        out=sl[:, :ck, :],
                        in0=dst_sb[:, int(roff[r]) : int(roff[r]) + ck][
                            :, :, None
                        ].to_broadcast([P, ck, W]),
                        in1=iot[:, None, :].to_broadcast([P, ck, W]),
                        op=mybir.AluOpType.is_equal,
                    )
                    for b in range(ck):
                        nc.tensor.matmul(
                            out=ps[:],
                            lhsT=mg[:, b0 + b, :],
                            rhs=sl[:, b, :],
                            start=(k == 0 and b == 0),
                            stop=(k == n_chunks - 1 and b == ck - 1),
                        )
                sg = stg[i]
                nc.scalar.copy(sg[:], ps[:])
                nc.sync.dma_start(out=out[:, w * W : (w + 1) * W], in_=sg[:])
    nc.compile()
    return nc


def compute_caps(src, dst, n_cores, nodes_per_core, chunk, n_chunks, n_win):
    """caps[w, k]: blocks needed by the fullest core's (w, k) region."""
    src = np.asarray(src, dtype=np.int64)
    dst = np.asarray(dst, dtype=np.int64)
    core = dst // nodes_per_core
    w = (dst - core * nodes_per_core) // W
    k = src // chunk
    rid = (core * n_win + w) * n_chunks + k
    counts = np.bincount(rid, minlength=n_cores * n_win * n_chunks)
    per_wk = counts.reshape(n_cores, n_win * n_chunks).max(axis=0)
    return np.maximum(-(-per_wk // P), 1).reshape(n_win, n_chunks)


def prep_core(src, dst, core, cap, n_win, nodes_per_core, chunk, n_chunks):
    """Bin one core's edges into (window, chunk) region planes.

    Slot order inside a region: ascending src (HBM row locality); real
    edges first, then idx-0 pads. Returns idxT [128, tot/16] int16
    (gather plane: slot s -> partition s%16 replicated over the 8 Q7
    groups, col s/16) and dstT [128, tot/128] int16 (slot s -> partition
    s%128, block s/128; window-relative dst, -1 on pads).
    """
    caps = np.asarray(cap)
    n_reg = n_win * n_chunks
    creg = caps.reshape(n_reg)
    roff = np.zeros(n_reg + 1, dtype=np.int64)
    roff[1:] = np.cumsum(creg)
    tot = int(roff[-1]) * P

    lo = core * nodes_per_core
    m = (dst >= lo) & (dst < lo + nodes_per_core)
    es = src[m].astype(np.int64)
    ed = (dst[m] - lo).astype(np.int64)
    w = ed // W
    k = es // chunk
    srel = es - k * chunk
    drel = ed - w * W
    rid = (w * n_chunks + k).astype(np.int64)

    order = np.argsort(rid, kind="stable")  # unsorted within region: sorted-by-src caused HBM bank conflicts across the 16 SDMA engines
    rid, srel, drel = rid[order], srel[order], drel[order]
    counts = np.bincount(rid, minlength=n_reg)
    starts = np.zeros(n_reg, dtype=np.int64)
    starts[1:] = np.cumsum(counts)[:-1]
    pos = np.arange(len(rid)) - starts[rid]
    slot = roff[rid] * P + pos

    idxflat = np.zeros(tot, np.int16)  # pad slots gather row 0 of their chunk
    dstflat = np.full(tot, -1, np.int16)
    idxflat[slot] = srel
    dstflat[slot] = drel
    idx16 = idxflat.reshape(tot // 16, 16).T  # [16, tot/16]
    idxT = np.ascontiguousarray(np.tile(idx16, (8, 1)))
    dstT = np.ascontiguousarray(dstflat.reshape(tot // P, P).T)
    return idxT, dstT


_cache = {}


def kernel(x, edge_index):
    x = np.asarray(x, dtype=np.float32)
    edge_index = np.asarray(edge_index)
    src = edge_index[0].astype(np.int64)
    dst = edge_index[1].astype(np.int64)

    n_win = -(-NODES_PER_CORE // W)  # 25
    caps = compute_caps(src, dst, N_CORES, NODES_PER_CORE, CHUNK, N_CHUNKS, n_win)

    key = (N_NODES, n_win, caps.tobytes())
    if key not in _cache:
        _cache[key] = build_program(N_NODES, CHUNK, N_CHUNKS, n_win, caps, N_CORES)
    nc = _cache[key]

    xp = np.ascontiguousarray(x.astype(ml_dtypes.bfloat16))
    iota = np.tile(np.arange(W, dtype=np.int16), (P, 1))
    in_maps = []
    for c in range(N_CORES):
        idxT, dstT = prep_core(
            src, dst, c, caps, n_win, NODES_PER_CORE, CHUNK, N_CHUNKS
        )
        in_maps.append({"xp": xp, "idxT": idxT, "dstT": dstT, "iota": iota})

    trace = bool(int(os.environ.get("KERNEL_TRACE", "0")))
    res = run_bass_kernel_spmd(
        nc, in_maps, core_ids=list(range(N_CORES)), trace=trace
    )
    if trace:
        kernel.last_results = res
    outs = [
        np.ascontiguousarray(res.results[c]["out"][:, :NODES_PER_CORE].T)
        for c in range(N_CORES)
    ]
    return np.ascontiguousarray(np.concatenate(outs, axis=0))
